# revision 28
# baseline (speedup 1.0000x reference)
"""Trainium2 Bass kernel for nn_AttentionScore (sparse local attention scores).

Reference computation (B=4, C=64, N=16384, S=16):
    tmp   = xyz[:, :, :, None] - neighbor_xyz            # [B,3,N,S]
    pos   = concat([tmp, ||tmp||], axis=1)               # [B,4,N,S]
    k     = Wk @ (neighbor_points + Wpos @ pos + bpos)   # [B,C,N,S]
    attn  = softmax_s((points*scale) . k)                # [B,N,S]

Softmax over s is shift-invariant, so every term constant in s drops out:
    attn[m,s] ~ sum_c qW[c,m]*np[c,m,s] + sum_j qp[j,m]*tmp[j,m,s] + qp3[m]*||tmp||
with qW[ci,m] = sum_co scale*Wk[co,ci]*points[co,m], qp = (scale*Wk@Wpos)^T @ points
(bpos and the xyz.qp dot cancel).

Sharding: N split contiguously across 8 cores (no communication needed).
m = b*2048 + n_local in [0, 8192) per core; halves h = m // 4096.

All bulk inputs are host-cast to bf16 (output tolerance is 2e-2; measured
error stays ~1e-3), halving HBM traffic. The main-term multiply uses an
(mq, s, mi) column order so the broadcast-qW operand has an innermost
unit-stride AP dim -> DVE 2x_1p bf16 mode. The channel reduction and the
positional j-reduction both run on TensorE with 0/1 selector matrices
shipped from the host; their PSUM rows land directly on softmax-layout
partitions so PSUM->SBUF copies never shift partitions.

Layouts per core (m = row index in [0, 8192), h = m//4096):
  NP  [128, 65536] bf16: part h*64+c, col t*8192 + mq*1024 + s*64 + mi
      where within half: mm = t*512 + mq*64 + mi   (8 supertiles)
  NX  [96, 4096]  bf16: part j*32 + (m//64)%32, col (m//2048)*1024 + s*64 + m%64
  XYZ [96, 256]   bf16: part j*32 + (m//64)%32, col (m//2048)*64 + m%64
  P   [64, 8192]  bf16: points[c, m]
  HS/SELN/SELA: 0/1 selector matrices (see make_in_maps)
  OUT [128, 1024] bf16: part m//64, col s*64 + m%64
"""

import sys

sys.path.insert(0, "/opt/trn_rl_repo")

import numpy as np
import ml_dtypes

import concourse.bass as bass
import concourse.bacc as bacc
import concourse.tile as tile
from concourse import mybir
from concourse.bass_utils import run_bass_kernel_spmd

F32 = mybir.dt.float32
BF16 = mybir.dt.bfloat16
AF = mybir.ActivationFunctionType
AX = mybir.AxisListType

BF = ml_dtypes.bfloat16

B, C, N, S = 4, 64, 16384, 16
NCORES = 8
NL = N // NCORES            # 2048 points per core
M = B * NL                  # 8192 (b, n) rows per core
MH = M // 2                 # 4096 rows per half
# Variable supertile sizes (mm per half): big tiles early for 32KB DMA
# descriptors (near-peak SDMA rate), small tiles last so the post-stream
# serial chain (mul -> matmuls -> scatter -> softmax) is short.
MBS = [1024, 1024, 1024, 512, 256, 256]
OFFS = [sum(MBS[:i]) for i in range(len(MBS))]
ST = len(MBS)
SCALE = float(C) ** -0.5


DEBUG = False


def _body(tc):
    nc = tc.nc

    NP = nc.dram_tensor("NP", [128, MH * S], BF16, kind="ExternalInput").ap()
    NX = nc.dram_tensor("NX", [96, 4096], BF16, kind="ExternalInput").ap()
    XYZ = nc.dram_tensor("XYZ", [96, 256], BF16, kind="ExternalInput").ap()
    P = nc.dram_tensor("P", [C, M], BF16, kind="ExternalInput").ap()
    WK = nc.dram_tensor("WK", [C, C], F32, kind="ExternalInput").ap()
    WKT = nc.dram_tensor("WKT", [C, C], F32, kind="ExternalInput").ap()
    WP = nc.dram_tensor("WP", [C, 4], F32, kind="ExternalInput").ap()
    HS = nc.dram_tensor("HS", [128, 2048], BF16, kind="ExternalInput").ap()
    SELN = nc.dram_tensor("SELN", [96, 128], BF16, kind="ExternalInput").ap()
    SELA = nc.dram_tensor("SELA", [128, 512], BF16, kind="ExternalInput").ap()
    OUT = nc.dram_tensor("OUT", [128, 1024], BF16, kind="ExternalOutput").ap()

    with (
        tc.tile_pool(name="const", bufs=1) as cp,
        tc.tile_pool(name="small", bufs=1) as sp,

        tc.tile_pool(name="tmp4", bufs=4) as tmpp,
        tc.tile_pool(name="sq2", bufs=2) as sqp,
        tc.tile_pool(name="pp2", bufs=2) as ppp,
        tc.tile_pool(name="npt", bufs=3) as npp,
        tc.tile_pool(name="sc", bufs=2) as scp,
        tc.tile_pool(name="soft", bufs=1) as smp,
        tc.tile_pool(name="psq", bufs=2, space="PSUM") as psq,
        tc.tile_pool(name="psn", bufs=2, space="PSUM") as psn,
        tc.tile_pool(name="psa", bufs=1, space="PSUM") as psa,
        tc.tile_pool(name="psm", bufs=3, space="PSUM") as psm,
    ):
        # small constants on the (otherwise idle) scalar HWDGE queue
        xyzt = cp.tile([96, 256], BF16)
        nc.scalar.dma_start(xyzt[:], XYZ)
        wk = cp.tile([C, C], F32)
        nc.scalar.dma_start(wk[:], WK)
        wkt = cp.tile([C, C], F32)
        nc.scalar.dma_start(wkt[:], WKT)
        wp = cp.tile([C, 4], F32)
        nc.scalar.dma_start(wp[:], WP)
        hs = cp.tile([128, 2048], BF16)
        nc.scalar.dma_start(hs[:], HS)
        seln = cp.tile([96, 128], BF16)
        nc.scalar.dma_start(seln[:], SELN)
        sela = cp.tile([128, 512], BF16)
        nc.scalar.dma_start(sela[:], SELA)

        # ---- ALL bulk loads on the single SWDGE ring: it drains strict-FIFO,
        # so the small latency-critical loads (pt, nxt) complete before the
        # 16.8MB NP stream, and each NP tile's completion semaphore fires as
        # its own descriptors finish instead of when the whole burst drains.
        pt = cp.tile([C, M], BF16)
        nc.gpsimd.dma_start(pt[:], P)
        nxt = cp.tile([96, 4096], BF16)
        nc.gpsimd.dma_start(nxt[:], NX)
        npts = []
        for t in range(ST):
            tag = ["npA", "npB", "npC", "npD", "npE", "npE"][t]
            npt = npp.tile([128, MBS[t] * S], BF16, name=f"np{t}", tag=tag,
                           bufs=2 if tag == "npE" else 1)
            nc.gpsimd.dma_start(npt[:], NP[:, OFFS[t] * S:(OFFS[t] + MBS[t]) * S])
            npts.append(npt)

        qw = cp.tile([128, MH], BF16)      # row h*64+c: qW[c, h*MH + mm]
        qpt2 = cp.tile([128, 256], BF16)   # row j*32+mbq: qp[j, (t4*32+mbq)*64+mi]
        attn1 = cp.tile([128, 1024], F32)  # part m//64, col s*64 + m%64
        attn2 = cp.tile([128, 1024], F32)

        # ---- phase 1: qW / qp, one bf16 matmul per 512-m chunk ----
        # wl_h [64, 128]: cols h*64..+64 = scale*Wk (-> qW rows), cols
        # (1-h)*64..+4 = scale*Wk@Wpos (-> qp rows). Both output row groups
        # land on the partitions their consumers read, so the PSUM->SBUF
        # copies are partition-aligned.
        wks = sp.tile([C, C], F32)
        nc.vector.tensor_scalar_mul(wks[:], wk[:], SCALE)
        wkts = sp.tile([C, C], F32)
        nc.vector.tensor_scalar_mul(wkts[:], wkt[:], SCALE)
        pwkp = psa.tile([C, 4], F32, tag="pa")
        nc.tensor.matmul(pwkp[:], lhsT=wkts[:], rhs=wp[:], start=True, stop=True)
        wkp = sp.tile([C, 4], F32)
        nc.scalar.copy(wkp[:], pwkp[:])

        wl = []
        for h in (0, 1):
            wlf = sp.tile([C, 128], F32, name=f"wlf{h}")
            nc.vector.memset(wlf[:], 0.0)
            nc.vector.tensor_copy(wlf[:, h * 64:(h + 1) * 64], wks[:])
            nc.vector.tensor_copy(wlf[:, (1 - h) * 64:(1 - h) * 64 + 4], wkp[:])
            wlb = sp.tile([C, 128], BF16, name=f"wl{h}")
            nc.vector.tensor_copy(wlb[:], wlf[:])
            wl.append(wlb)

        # qp staging: h=1 qp rows land on partitions 0..4, h=0 on 64..68;
        # columns are m_loc within the half. Scattered to qpt2 afterwards.
        qps = cp.tile([68, MH], BF16)
        for cc in range(8):
            for h in range(2):
                pq = psq.tile([128, 512], F32)
                csl = slice(h * MH + cc * 512, h * MH + (cc + 1) * 512)
                nc.tensor.matmul(pq[:], lhsT=wl[h][:], rhs=pt[:, csl], start=True, stop=True)
                nc.scalar.copy(qw[h * 64:(h + 1) * 64, cc * 512:(cc + 1) * 512],
                               pq[h * 64:(h + 1) * 64, :])
                r0 = (1 - h) * 64
                nc.scalar.copy(qps[r0:r0 + 4, cc * 512:(cc + 1) * 512],
                               pq[r0:r0 + 4, :])
        # scatter: qpt2[j*32 + b%32, (2h + b//32)*64 + mi] = qp[j, h*MH + b*64 + mi]
        # (scalar HWDGE: off the SWDGE ring so it isn't queued behind NP)
        for h in range(2):
            for j in range(4):
                for b2 in range(2):
                    t4 = 2 * h + b2
                    nc.scalar.dma_start(
                        qpt2[j * 32:(j + 1) * 32, t4 * 64:(t4 + 1) * 64],
                        qps[(1 - h) * 64 + j:(1 - h) * 64 + j + 1,
                            b2 * 2048:(b2 + 1) * 2048]
                        .rearrange("p (q mi) -> p q mi", q=32, mi=64),
                    )

        # ---- phase 2a: tmp = xyz - nx, norm via TensorE selector reduce ----
        # tmp4[t4] rows: j*32+mbq = tmp_j for m-block t4*32+mbq; rows 96..128
        # get ||tmp|| (sqrt writes straight from PSUM rows 96..128).
        tmps = []
        for t4 in range(4):
            csl = slice(t4 * 1024, (t4 + 1) * 1024)
            tmp = tmpp.tile([128, 1024], BF16, name=f"tmp{t4}", tag="tmp")
            nc.vector.tensor_sub(
                tmp[0:96, :].rearrange("p (s mi) -> p s mi", s=S),
                xyzt[:, t4 * 64:(t4 + 1) * 64]
                .rearrange("p (one mi) -> p one mi", one=1)
                .broadcast_to((96, S, 64)),
                nxt[:, csl].rearrange("p (s mi) -> p s mi", s=S),
            )
            sq = sqp.tile([96, 1024], BF16)
            nc.scalar.square(sq[:], tmp[0:96, :])
            for half in range(2):
                pn = psn.tile([128, 512], F32)
                nc.tensor.matmul(pn[:], lhsT=seln[:],
                                 rhs=sq[:, half * 512:(half + 1) * 512],
                                 start=True, stop=True)
                nc.scalar.sqrt(tmp[96:128, half * 512:(half + 1) * 512],
                               pn[96:128, :])
            tmps.append(tmp)

        # ---- phase 3: main term supertiles (+ interleaved pos term) ----
        for t in range(ST):
            npt = npts[t]
            mb = MBS[t]
            mq = mb // 64
            ch = mb * S // 512  # 512-col PE chunks in this supertile
            qwb = (
                qw[:, OFFS[t]:OFFS[t] + mb]
                .rearrange("p (mq one mi) -> p mq one mi", mq=mq, one=1, mi=64)
                .broadcast_to((128, mq, S, 64))
            )
            # in-place: npt <- npt * qW (saves an 8MB prod pool; DVE reads
            # each element before its write reaches it)
            nptv = npt[:].rearrange("p (mq s mi) -> p mq s mi", mq=mq, s=S)
            nc.vector.tensor_mul(nptv, nptv, qwb)
            ps = psm.tile([64, 512], F32)
            for k in range(ch):
                nc.tensor.matmul(
                    ps[:],
                    lhsT=hs[:, k * 64:(k + 1) * 64],
                    rhs=npt[:, k * 512:(k + 1) * 512],
                    start=(k == 0),
                    stop=(k == ch - 1),
                )
            sc = scp.tile([64, 512], F32)
            nc.scalar.copy(sc[:], ps[:])
            # row h*32+k holds (mq=k//2, s-half=k%2) -> partition
            # h*64 + OFFS[t]//64 + k//2, col (k%2)*512 + si*64 + mi:
            # 2KB-contiguous per row.
            p0 = OFFS[t] // 64
            seng = nc.sync
            for h in range(2):
                seng.dma_start(
                    attn1[h * 64 + p0:h * 64 + p0 + ch // 2, :]
                    .rearrange("p (k1 f) -> p k1 f", k1=2),
                    sc[h * 32:h * 32 + ch, :],
                )

            # phase 2b interleaved: pos products + TensorE j-reduce for one
            # m-quarter. Fills the DVE/PE gaps while NP streams.
            if t < 4:
                t4 = t
                tmp = tmps[t4]
                pp = ppp.tile([128, 1024], BF16)
                nc.vector.tensor_mul(
                    pp[:].rearrange("p (s mi) -> p s mi", s=S),
                    tmp[:].rearrange("p (s mi) -> p s mi", s=S),
                    qpt2[:, t4 * 64:(t4 + 1) * 64]
                    .rearrange("p (one mi) -> p one mi", one=1)
                    .broadcast_to((128, S, 64)),
                )
                for half in range(2):
                    pa = psa.tile([128, 512], F32, tag="pa")
                    nc.tensor.matmul(pa[:], lhsT=sela[:, t4 * 128:(t4 + 1) * 128],
                                     rhs=pp[:, half * 512:(half + 1) * 512],
                                     start=True, stop=True)
                    nc.scalar.copy(
                        attn2[t4 * 32:(t4 + 1) * 32, half * 512:(half + 1) * 512],
                        pa[t4 * 32:(t4 + 1) * 32, :])

        # ---- phase 4: softmax over s (|attn| < ~6, so no max-subtract) ----
        nc.vector.tensor_add(attn1[:], attn1[:], attn2[:])
        e = smp.tile([128, 1024], BF16)
        nc.scalar.activation(e[:], attn1[:], AF.Exp)
        se = smp.tile([128, 64], F32)
        nc.vector.reduce_sum(
            se[:], e[:].rearrange("p (s mi) -> p mi s", s=S), axis=AX.X
        )
        rse = smp.tile([128, 64], F32)
        nc.vector.reciprocal(rse[:], se[:])
        rse16 = smp.tile([128, 64], BF16)
        nc.vector.tensor_copy(rse16[:], rse[:])
        ev = e[:].rearrange("p (s mi) -> p s mi", s=S)
        nc.vector.tensor_mul(
            ev, ev,
            rse16[:].rearrange("p (one mi) -> p one mi", one=1)
            .broadcast_to((128, S, 64)),
        )
        nc.sync.dma_start(OUT, e[:])

        if DEBUG:
            DQW = nc.dram_tensor("DQW", [128, MH], BF16, kind="ExternalOutput").ap()
            DQP = nc.dram_tensor("DQP", [128, 256], BF16, kind="ExternalOutput").ap()
            DA1 = nc.dram_tensor("DA1", [128, 1024], F32, kind="ExternalOutput").ap()
            DA2 = nc.dram_tensor("DA2", [128, 1024], F32, kind="ExternalOutput").ap()
            DT0 = nc.dram_tensor("DT0", [128, 1024], BF16, kind="ExternalOutput").ap()
            nc.scalar.dma_start(DQW, qw[:])
            nc.scalar.dma_start(DQP, qpt2[:])
            nc.scalar.dma_start(DA1, attn1[:])
            nc.scalar.dma_start(DA2, attn2[:])
            nc.scalar.dma_start(DT0, tmps[0][:])


_NC_CACHE = None


def build_nc():
    global _NC_CACHE
    if _NC_CACHE is None:
        nc = bacc.Bacc(trn_type="TRN2", target_bir_lowering=False, debug=False)
        with tile.TileContext(nc) as tc:
            _body(tc)
        nc.compile()
        _NC_CACHE = nc
    return _NC_CACHE


def _selectors():
    # HS [128, 2048]: hs[p, k*64 + r] = 1 iff r == (p//64)*32 + k
    hs = np.zeros((128, 2048), dtype=BF)
    for k in range(32):
        for h in range(2):
            hs[h * 64:(h + 1) * 64, k * 64 + h * 32 + k] = 1
    # SELN [96, 128]: col 96+q sums partitions {q, 32+q, 64+q}
    seln = np.zeros((96, 128), dtype=BF)
    for q in range(32):
        for j in range(3):
            seln[j * 32 + q, 96 + q] = 1
    # SELA [128, 512]: col t4*128 + r (r in [t4*32, t4*32+32)) sums
    # partitions {r%32 + 32j : j in 0..4}
    sela = np.zeros((128, 512), dtype=BF)
    for t4 in range(4):
        for q in range(32):
            for j in range(4):
                sela[j * 32 + q, t4 * 128 + t4 * 32 + q] = 1
    return hs, seln, sela


def make_in_maps(xyz, neighbor_xyz, points, neighbor_points, Wk, Wpos, bpos):
    """Slice + relayout full inputs into the 8 per-core input maps."""
    xyz = np.asarray(xyz, dtype=np.float32)
    neighbor_xyz = np.asarray(neighbor_xyz, dtype=np.float32)
    points = np.asarray(points, dtype=np.float32)
    neighbor_points = np.asarray(neighbor_points, dtype=np.float32)
    Wk = np.ascontiguousarray(np.asarray(Wk, dtype=np.float32))
    WkT = np.ascontiguousarray(Wk.T)
    Wp = np.ascontiguousarray(np.asarray(Wpos, dtype=np.float32))
    hs, seln, sela = _selectors()

    in_maps = []
    for i in range(NCORES):
        nsl = slice(i * NL, (i + 1) * NL)
        # np: [B,C,nl,S] -> [c, m, s] -> per supertile (h, c, mq, s, mi),
        # concatenated along columns -> [128, 65536]
        npc = neighbor_points[:, :, nsl, :].transpose(1, 0, 2, 3).reshape(C, M, S)
        npc = npc.reshape(C, 2, MH, S).transpose(1, 0, 2, 3)  # (h, c, mm, s)
        blocks = []
        for t in range(ST):
            blk = npc[:, :, OFFS[t]:OFFS[t] + MBS[t], :]
            blk = (
                blk.reshape(2, C, MBS[t] // 64, 64, S)
                .transpose(0, 1, 2, 4, 3)
                .reshape(128, MBS[t] * S)
            )
            blocks.append(blk)
        npc = np.concatenate(blocks, axis=1)
        # nx: [B,3,nl,S] -> [j, m, s] -> (j, mbq, t4, s, mi) -> [96, 4096]
        nxc = (
            neighbor_xyz[:, :, nsl, :]
            .transpose(1, 0, 2, 3)
            .reshape(3, M, S)
            .reshape(3, 4, 32, 64, S)
            .transpose(0, 2, 1, 4, 3)
            .reshape(96, 4096)
        )
        # xyz: [B,3,nl] -> (j, mbq, t4, mi) -> [96, 256]
        xc = (
            xyz[:, :, nsl]
            .transpose(1, 0, 2)
            .reshape(3, M)
            .reshape(3, 4, 32, 64)
            .transpose(0, 2, 1, 3)
            .reshape(96, 256)
        )
        # points: [B,C,nl] -> [c, m]
        pc = points[:, :, nsl].transpose(1, 0, 2).reshape(C, M)
        in_maps.append(
            {
                "NP": np.ascontiguousarray(npc.astype(BF)),
                "NX": np.ascontiguousarray(nxc.astype(BF)),
                "XYZ": np.ascontiguousarray(xc.astype(BF)),
                "P": np.ascontiguousarray(pc.astype(BF)),
                "WK": Wk,
                "WKT": WkT,
                "WP": Wp,
                "HS": hs,
                "SELN": seln,
                "SELA": sela,
            }
        )
    return in_maps


def assemble_output(results):
    """Per-core OUT [128, 1024] bf16 (p, s, mi) -> full [B, N, S] f32."""
    out = np.empty((B, N, S), dtype=np.float32)
    for i in range(NCORES):
        oc = np.asarray(results[i]["OUT"]).astype(np.float32)
        oc = oc.reshape(128, S, 64).transpose(0, 2, 1).reshape(M, S)
        out[:, i * NL:(i + 1) * NL, :] = oc.reshape(B, NL, S)
    return out


def run_cores(in_maps, trace=False, trace_kwargs=None):
    nc = build_nc()
    return run_bass_kernel_spmd(
        nc,
        in_maps,
        core_ids=list(range(NCORES)),
        trace=trace,
        **(trace_kwargs or {}),
    )


def kernel(xyz, neighbor_xyz, points, neighbor_points, Wk, Wpos, bpos):
    in_maps = make_in_maps(
        xyz, neighbor_xyz, points, neighbor_points, Wk, Wpos, bpos
    )
    res = run_cores(in_maps, trace=False)
    return assemble_output(res.results)
